# revision 20
# baseline (speedup 1.0000x reference)
"""DDAU encoder (3-layer noisy GNN message passing) on 8 trn2 NeuronCores.

Strategy (1D row sharding):
  - Core k owns output rows [k*18750, (k+1)*18750).
  - Host sorts each core's edges by (source col-block, dest row-block) and
    pads each (col-block, row-block) segment to a multiple of 128 edges with
    a cross-core-uniform tile count, so one SPMD program serves all cores.
  - Per 128-edge tile: dma_gather pulls x[col] rows from HBM (edge on
    partition), a fused DVE tensor_scalar builds H[e,m] = (m==lrow[e])*val[e],
    and the tensor engine accumulates H^T @ G into the 128-row output block
    in PSUM. No scatter DMA anywhere.
  - Per layer: noise injection epilogue on the owned slice, then an HBM
    AllGather shares each core's updated slice for the next layer's gathers.
"""

import numpy as np

N = 150000
USER_NUM = 100000
NCORES = 8
RPC = N // NCORES          # 18750 rows per core
EMB = 64
NRB = (RPC + 127) // 128   # 147 row blocks; last block has 62 rows
FULL_RB = RPC // 128       # 146 full blocks
TAIL_R = RPC - FULL_RB * 128  # 62
NCB = NCORES               # 8 col blocks of RPC rows each (idx fits int16)
NL = 3
EPS = 0.1
CH_T = 64                  # gather chunk size in tiles (64*128 idxs/call)
EP_C = 21                  # epilogue row-block chunk (7 chunks of 21)


def _preprocess(adj_rows, adj_cols, adj_vals):
    rows = np.asarray(adj_rows).astype(np.int64)
    cols = np.asarray(adj_cols).astype(np.int64)
    vals = np.asarray(adj_vals).astype(np.float32)

    core = rows // RPC
    lr = rows - core * RPC
    rb = lr >> 7
    lrow128 = (lr & 127).astype(np.float32)
    cbv = cols // RPC
    lcol = (cols - cbv * RPC).astype(np.int16)

    per_core = []
    counts = np.zeros((NCORES, NCB, NRB), np.int64)
    for k in range(NCORES):
        m = core == k
        key = (cbv[m] * NRB + rb[m]).astype(np.int64)
        order = np.argsort(key, kind="stable")
        per_core.append((lcol[m][order], lrow128[m][order], vals[m][order]))
        counts[k] = np.bincount(key, minlength=NCB * NRB).reshape(NCB, NRB)

    seg_tiles = -(-counts.max(axis=0) // 128)          # [NCB, NRB]
    T_tot = int(seg_tiles.sum())

    seg_offsets = np.zeros((NCB, NRB), np.int64)
    toff = 0
    for c in range(NCB):
        for b in range(NRB):
            seg_offsets[c, b] = toff
            toff += int(seg_tiles[c, b])

    chunks = []
    for c in range(NCB):
        start = int(seg_offsets[c, 0])
        end = int(seg_offsets[c + 1, 0]) if c + 1 < NCB else T_tot
        t = start
        while t < end:
            n = min(CH_T, end - t)
            chunks.append((c, t, n))
            t += n

    streams = []
    for k in range(NCORES):
        lc, lrw, vl = per_core[k]
        E_pad = T_tot * 128
        lcol_s = np.zeros(E_pad, np.int16)
        lrow_s = np.full(E_pad, -1.0, np.float32)
        vals_s = np.zeros(E_pad, np.float32)
        src_ofs = np.zeros(NCB * NRB + 1, np.int64)
        np.cumsum(counts[k].reshape(-1), out=src_ofs[1:])
        for c in range(NCB):
            for b in range(NRB):
                s = int(src_ofs[c * NRB + b])
                e = int(src_ofs[c * NRB + b + 1])
                d = int(seg_offsets[c, b]) * 128
                lcol_s[d:d + e - s] = lc[s:e]
                lrow_s[d:d + e - s] = lrw[s:e]
                vals_s[d:d + e - s] = vl[s:e]
        streams.append({
            "idx": np.tile(lcol_s.reshape(-1, 16).T, (8, 1)),        # [128,T*8]
            "lrow": np.ascontiguousarray(lrow_s.reshape(-1, 128).T),  # [128,T]
            "vals": np.ascontiguousarray(vals_s.reshape(-1, 128).T),
        })
    return seg_tiles, seg_offsets, chunks, T_tot, streams


def _build_program(seg_tiles, seg_offsets, chunks, T_tot, mode="full"):
    import concourse.bacc as bacc
    import concourse.mybir as mybir
    import concourse.tile as tile
    from concourse.library_config import mlp

    f32 = mybir.dt.float32
    i16 = mybir.dt.int16
    i32 = mybir.dt.int32
    Alu = mybir.AluOpType

    nc = bacc.Bacc("TRN2", target_bir_lowering=False, debug=False,
                   num_devices=NCORES, num_swdge_queues=4)
    x0 = nc.dram_tensor("x0", [N, EMB], f32, kind="ExternalInput")
    idx_d = nc.dram_tensor("idx", [128, T_tot * 8], i16, kind="ExternalInput")
    lrw_d = nc.dram_tensor("lrw", [128, T_tot], f32, kind="ExternalInput")
    vls_d = nc.dram_tensor("vls", [128, T_tot], f32, kind="ExternalInput")
    noise_d = nc.dram_tensor("noise", [NL, RPC, EMB], f32, kind="ExternalInput")
    out_d = nc.dram_tensor("out", [RPC, EMB], f32, kind="ExternalOutput")
    ego_d = nc.dram_tensor("ego_slice", [RPC, EMB], f32)
    xg = [nc.dram_tensor(f"xg{i}", [N, EMB], f32, addr_space="Shared")
          for i in range(2)]

    # chunk lookup: tile index -> chunk (they are in increasing toff order)
    chunk_start_of = {}
    for ci, (c, toff, nt) in enumerate(chunks):
        chunk_start_of[toff] = ci

    with tile.TileContext(nc) as tc:
        nc.gpsimd.load_library(mlp)
        with tc.tile_pool(name="const", bufs=1) as constp, \
             tc.tile_pool(name="big", bufs=1) as big, \
             tc.tile_pool(name="gp", bufs=3) as gp, \
             tc.tile_pool(name="mp", bufs=3) as mp, \
             tc.tile_pool(name="hp", bufs=6) as hp, \
             tc.tile_pool(name="ep", bufs=2) as ep, \
             tc.tile_pool(name="pp", bufs=8, space="PSUM") as pp:

            iota_i = constp.tile([128, 128], i32)
            iota_f = constp.tile([128, 128], f32)
            nc.gpsimd.iota(iota_i[:], pattern=[[1, 128]], base=0,
                           channel_multiplier=0)
            nc.vector.tensor_copy(iota_f[:], iota_i[:])

            acc = big.tile([128, NRB, EMB], f32)
            spmm = big.tile([128, NRB, EMB], f32)
            nzb = big.tile([128, NRB, EMB], f32)
            n2 = big.tile([128, NRB], f32)
            nrm = big.tile([128, NRB], f32)
            rinv = big.tile([128, NRB], f32)

            nc.vector.memset(acc[:], 0.0)
            nc.vector.memset(nzb[:], 0.0)

            for layer in range(NL):
                src_t = [x0, xg[0], xg[1]][layer] if mode == "full" else x0
                nc.vector.memset(spmm[:], 0.0)

                # ---- noise prep (independent of spmm; overlaps gathers) ----
                nc.sync.dma_start(
                    nzb[:, :FULL_RB, :],
                    noise_d[layer, :FULL_RB * 128, :]
                    .rearrange("(b p) d -> p b d", p=128))
                nc.sync.dma_start(nzb[0:TAIL_R, FULL_RB, :],
                                  noise_d[layer, FULL_RB * 128:, :])
                for c0 in range(0, NRB, EP_C):
                    n = min(EP_C, NRB - c0)
                    sq = ep.tile([128, EP_C, EMB], f32, tag="sq")
                    nc.vector.tensor_tensor(
                        out=sq[:, :n, :], in0=nzb[:, c0:c0 + n, :],
                        in1=nzb[:, c0:c0 + n, :], op=Alu.mult)
                    nc.vector.tensor_reduce(
                        out=n2[:, c0:c0 + n], in_=sq[:, :n, :],
                        axis=mybir.AxisListType.X, op=Alu.add)
                nc.scalar.sqrt(nrm[:, :], n2[:, :])
                nc.vector.reciprocal(rinv[:, :], nrm[:, :])
                for b in range(NRB):
                    nc.vector.tensor_scalar(
                        out=nzb[:, b, :], in0=nzb[:, b, :],
                        scalar1=rinv[:, b:b + 1], scalar2=float(EPS),
                        op0=Alu.mult, op1=Alu.mult)

                def epilogue_chunk(c0, n):
                    # ego = spmm + sign(spmm)*nn ; acc += ego ; store slice
                    sg = ep.tile([128, EP_C, EMB], f32, tag="sg")
                    nc.scalar.sign(sg[:, :n, :], spmm[:, c0:c0 + n, :])
                    nc.vector.tensor_tensor(
                        out=sg[:, :n, :], in0=sg[:, :n, :],
                        in1=nzb[:, c0:c0 + n, :], op=Alu.mult)
                    nc.vector.tensor_tensor(
                        out=spmm[:, c0:c0 + n, :], in0=spmm[:, c0:c0 + n, :],
                        in1=sg[:, :n, :], op=Alu.add)
                    nc.vector.tensor_tensor(
                        out=acc[:, c0:c0 + n, :], in0=acc[:, c0:c0 + n, :],
                        in1=spmm[:, c0:c0 + n, :], op=Alu.add)
                    if layer < NL - 1:
                        nfull = min(c0 + n, FULL_RB) - c0
                        if nfull > 0:
                            nc.sync.dma_start(
                                ego_d[c0 * 128:(c0 + nfull) * 128, :]
                                .rearrange("(b p) d -> p b d", p=128),
                                spmm[:, c0:c0 + nfull, :])
                        if c0 + n > FULL_RB:
                            nc.sync.dma_start(
                                ego_d[FULL_RB * 128:, :],
                                spmm[0:TAIL_R, FULL_RB, :])

                cur = {"g": None, "lrow": None, "vals": None, "start": -1,
                       "nt": 0}
                if mode == "compute_only":
                    dg = gp.tile([128, CH_T, EMB], f32, tag="g")
                    dl = mp.tile([128, CH_T], f32, tag="lrow")
                    dv = mp.tile([128, CH_T], f32, tag="vals")
                    nc.vector.memset(dg[:], 0.25)
                    nc.vector.memset(dl[:], 1.0)
                    nc.vector.memset(dv[:], 0.5)
                    cur.update(g=dg, lrow=dl, vals=dv, start=0)

                def begin_chunk(ci):
                    cbv, toff, nt = chunks[ci]
                    qn = ci % 4
                    idxc = mp.tile([128, CH_T * 8], i16, tag="idx")
                    nc.sync.dma_start(idxc[:, :nt * 8],
                                      idx_d[:, toff * 8:(toff + nt) * 8])
                    lrowc = mp.tile([128, CH_T], f32, tag="lrow")
                    nc.sync.dma_start(lrowc[:, :nt],
                                      lrw_d[:, toff:toff + nt])
                    valsc = mp.tile([128, CH_T], f32, tag="vals")
                    nc.sync.dma_start(valsc[:, :nt],
                                      vls_d[:, toff:toff + nt])
                    g = gp.tile([128, CH_T, EMB], f32, tag="g")
                    nc.gpsimd.dma_gather(
                        g[:, :nt, :],
                        src_t[cbv * RPC:(cbv + 1) * RPC, :],
                        idxc[:, :nt * 8], nt * 128, nt * 128, EMB,
                        single_packet=False, queue_num=qn)
                    cur["g"], cur["lrow"], cur["vals"] = g, lrowc, valsc
                    cur["start"], cur["nt"] = toff, nt

                do_dma = mode in ("full", "gather_only", "nocc")
                do_compute = mode in ("full", "compute_only", "nocc")
                for c in range(NCB):
                    for b in range(NRB):
                        n_t = int(seg_tiles[c, b])
                        if n_t > 0:
                            g0 = int(seg_offsets[c, b])
                            if do_compute:
                                ps = pp.tile([128, EMB], f32, space="PSUM",
                                             tag="ps")
                            for j in range(n_t):
                                t = g0 + j
                                if t in chunk_start_of and do_dma:
                                    begin_chunk(chunk_start_of[t])
                                if not do_compute:
                                    continue
                                col = (t - cur["start"]) if do_dma else 0
                                h = hp.tile([128, 128], f32, tag="h")
                                nc.vector.tensor_scalar(
                                    out=h[:], in0=iota_f[:],
                                    scalar1=cur["lrow"][:, col:col + 1],
                                    scalar2=cur["vals"][:, col:col + 1],
                                    op0=Alu.is_equal, op1=Alu.mult)
                                nc.tensor.matmul(
                                    out=ps[:], lhsT=h[:],
                                    rhs=cur["g"][:, col, :],
                                    start=(j == 0), stop=(j == n_t - 1))
                            if do_compute:
                                nc.vector.tensor_tensor(
                                    out=spmm[:, b, :], in0=spmm[:, b, :],
                                    in1=ps[:], op=Alu.add)
                        if c == NCB - 1 and ((b + 1) % EP_C == 0
                                             or b == NRB - 1):
                            c0 = (b // EP_C) * EP_C
                            if do_compute:
                                epilogue_chunk(c0, b - c0 + 1)

                if layer < NL - 1 and mode == "full":
                    nc.gpsimd.collective_compute(
                        "AllGather", mybir.AluOpType.bypass,
                        replica_groups=[list(range(NCORES))],
                        ins=[ego_d[:]], outs=[xg[layer][:]])

            nc.vector.tensor_scalar_mul(acc[:], acc[:], 1.0 / NL)
            nc.sync.dma_start(
                out_d[:FULL_RB * 128, :].rearrange("(b p) d -> p b d", p=128),
                acc[:, :FULL_RB, :])
            nc.sync.dma_start(out_d[FULL_RB * 128:, :],
                              acc[0:TAIL_R, FULL_RB, :])
    nc.compile()
    return nc


def _run(inputs, trace=False):
    from concourse.bass_utils import run_bass_kernel_spmd

    user_emb = np.asarray(inputs["user_emb"], dtype=np.float32)
    item_emb = np.asarray(inputs["item_emb"], dtype=np.float32)
    noise = np.asarray(inputs["noise"], dtype=np.float32)
    x0 = np.concatenate([user_emb, item_emb], axis=0)

    seg_tiles, seg_offsets, chunks, T_tot, streams = _preprocess(
        inputs["adj_rows"], inputs["adj_cols"], inputs["adj_vals"])

    nc = _build_program(seg_tiles, seg_offsets, chunks, T_tot)

    in_maps = []
    for k in range(NCORES):
        in_maps.append({
            "x0": x0,
            "idx": streams[k]["idx"],
            "lrw": streams[k]["lrow"],
            "vls": streams[k]["vals"],
            "noise": np.ascontiguousarray(noise[:, k * RPC:(k + 1) * RPC, :]),
        })
    res = run_bass_kernel_spmd(nc, in_maps, core_ids=list(range(NCORES)),
                               trace=trace)
    res._timing_ctx = (nc, in_maps)
    acc = np.concatenate([res.results[k]["out"] for k in range(NCORES)],
                         axis=0)
    user_all = acc[:USER_NUM]
    item_all = acc[USER_NUM:]
    outs = (user_all, item_all,
            np.asarray(inputs["user_prototypes"], dtype=np.float32),
            np.asarray(inputs["item_prototypes"], dtype=np.float32))
    return outs, res


def _time_neff(nc, in_maps, reps=5):
    """Wall-clock the NEFF execution with device-resident inputs.

    Mirrors bass2jax.run_bass_via_pjrt's multi-core path but without
    donation so the same device buffers can be re-executed."""
    import time

    import jax
    import numpy as np_
    from jax.sharding import Mesh, NamedSharding, PartitionSpec
    from jax.experimental.shard_map import shard_map

    import concourse.mybir as mybir
    from concourse import bass2jax

    bass2jax.install_neuronx_cc_hook()

    partition_name = (nc.partition_id_tensor.name
                      if nc.partition_id_tensor else None)
    in_names, out_names, out_avals, zero_outs = [], [], [], []
    for alloc in nc.m.functions[0].allocations:
        if not isinstance(alloc, mybir.MemoryLocationSet):
            continue
        name = alloc.memorylocations[0].name
        if alloc.kind == "ExternalInput":
            if name != partition_name:
                in_names.append(name)
        elif alloc.kind == "ExternalOutput":
            shape = tuple(alloc.tensor_shape)
            dtype = mybir.dt.np(alloc.dtype)
            out_names.append(name)
            out_avals.append(jax.core.ShapedArray(shape, dtype))
            zero_outs.append(np_.zeros(shape, dtype))
    n_params = len(in_names)
    all_names = in_names + out_names

    bind_names = list(all_names)
    if partition_name is not None:
        bind_names.append(partition_name)

    def _body(*args):
        operands = list(args)
        if partition_name is not None:
            operands.append(bass2jax.partition_id_tensor())
        outs = bass2jax._bass_exec_p.bind(
            *operands,
            out_avals=tuple(out_avals),
            in_names=tuple(bind_names),
            out_names=tuple(out_names),
            lowering_input_output_aliases=(),
            sim_require_finite=True,
            sim_require_nnan=True,
            nc=nc,
        )
        return tuple(outs)

    devices = jax.devices()[:NCORES]
    mesh = Mesh(np_.asarray(devices), ("core",))
    nspec = len(all_names)
    sharded = jax.jit(shard_map(
        _body, mesh=mesh, in_specs=(PartitionSpec("core"),) * nspec,
        out_specs=(PartitionSpec("core"),) * len(out_names), check_rep=False))

    sh = NamedSharding(mesh, PartitionSpec("core"))
    dev_args = []
    for i, name in enumerate(all_names):
        if i < n_params:
            arr = np_.concatenate(
                [np_.asarray(m[name]) for m in in_maps], axis=0)
        else:
            z = zero_outs[i - n_params]
            arr = np_.zeros((NCORES * z.shape[0], *z.shape[1:]), z.dtype)
        dev_args.append(jax.device_put(arr, sh))

    times = []
    for _ in range(reps):
        t0 = time.perf_counter()
        out = sharded(*dev_args)
        jax.block_until_ready(out)
        times.append(time.perf_counter() - t0)
    return times


def kernel(**inputs):
    outs, _ = _run(inputs, trace=False)
    return outs


# revision 21
# speedup vs baseline: 1.2982x; 1.2982x over previous
"""DDAU encoder (3-layer noisy GNN message passing) on 8 trn2 NeuronCores.

Strategy (1D row sharding):
  - Core k owns output rows [k*18750, (k+1)*18750).
  - Host sorts each core's edges by (source col-block, dest row-block) and
    pads each (col-block, row-block) segment to a multiple of 128 edges with
    a cross-core-uniform tile count, so one SPMD program serves all cores.
  - Per 128-edge tile: dma_gather pulls x[col] rows from HBM (edge on
    partition), a fused DVE tensor_scalar builds H[e,m] = (m==lrow[e])*val[e],
    and the tensor engine accumulates H^T @ G into the 128-row output block
    in PSUM. No scatter DMA anywhere.
  - Per layer: noise injection epilogue on the owned slice, then an HBM
    AllGather shares each core's updated slice for the next layer's gathers.
"""

import numpy as np

N = 150000
USER_NUM = 100000
NCORES = 8
RPC = N // NCORES          # 18750 rows per core
EMB = 64
NRB = (RPC + 127) // 128   # 147 row blocks; last block has 62 rows
FULL_RB = RPC // 128       # 146 full blocks
TAIL_R = RPC - FULL_RB * 128  # 62
NCB = NCORES               # 8 col blocks of RPC rows each (idx fits int16)
NL = 3
EPS = 0.1
CH_T = 64                  # gather chunk size in tiles (64*128 idxs/call)
EP_C = 21                  # epilogue row-block chunk (7 chunks of 21)


def _preprocess(adj_rows, adj_cols, adj_vals):
    rows = np.asarray(adj_rows).astype(np.int64)
    cols = np.asarray(adj_cols).astype(np.int64)
    vals = np.asarray(adj_vals).astype(np.float32)

    core = rows // RPC
    lr = rows - core * RPC
    rb = lr >> 7
    lrow128 = (lr & 127).astype(np.float32)
    cbv = cols // RPC
    lcol = (cols - cbv * RPC).astype(np.int16)

    per_core = []
    counts = np.zeros((NCORES, NCB, NRB), np.int64)
    for k in range(NCORES):
        m = core == k
        key = (cbv[m] * NRB + rb[m]).astype(np.int64)
        order = np.argsort(key, kind="stable")
        per_core.append((lcol[m][order], lrow128[m][order], vals[m][order]))
        counts[k] = np.bincount(key, minlength=NCB * NRB).reshape(NCB, NRB)

    seg_tiles = -(-counts.max(axis=0) // 128)          # [NCB, NRB]
    T_tot = int(seg_tiles.sum())

    seg_offsets = np.zeros((NCB, NRB), np.int64)
    toff = 0
    for c in range(NCB):
        for b in range(NRB):
            seg_offsets[c, b] = toff
            toff += int(seg_tiles[c, b])

    chunks = []
    for c in range(NCB):
        start = int(seg_offsets[c, 0])
        end = int(seg_offsets[c + 1, 0]) if c + 1 < NCB else T_tot
        t = start
        while t < end:
            n = min(CH_T, end - t)
            chunks.append((c, t, n))
            t += n

    streams = []
    for k in range(NCORES):
        lc, lrw, vl = per_core[k]
        E_pad = T_tot * 128
        lcol_s = np.zeros(E_pad, np.int16)
        lrow_s = np.full(E_pad, -1.0, np.float32)
        vals_s = np.zeros(E_pad, np.float32)
        src_ofs = np.zeros(NCB * NRB + 1, np.int64)
        np.cumsum(counts[k].reshape(-1), out=src_ofs[1:])
        for c in range(NCB):
            for b in range(NRB):
                s = int(src_ofs[c * NRB + b])
                e = int(src_ofs[c * NRB + b + 1])
                d = int(seg_offsets[c, b]) * 128
                lcol_s[d:d + e - s] = lc[s:e]
                lrow_s[d:d + e - s] = lrw[s:e]
                vals_s[d:d + e - s] = vl[s:e]
        streams.append({
            "idx": np.tile(lcol_s.reshape(-1, 16).T, (8, 1)),        # [128,T*8]
            "lrow": np.ascontiguousarray(lrow_s.reshape(-1, 128).T),  # [128,T]
            "vals": np.ascontiguousarray(vals_s.reshape(-1, 128).T),
        })
    return seg_tiles, seg_offsets, chunks, T_tot, streams


def _build_program(seg_tiles, seg_offsets, chunks, T_tot, mode="full"):
    import concourse.bacc as bacc
    import concourse.mybir as mybir
    import concourse.tile as tile
    from concourse.library_config import mlp

    f32 = mybir.dt.float32
    i16 = mybir.dt.int16
    i32 = mybir.dt.int32
    Alu = mybir.AluOpType

    nc = bacc.Bacc("TRN2", target_bir_lowering=False, debug=False,
                   num_devices=NCORES, num_swdge_queues=4)
    x0 = nc.dram_tensor("x0", [N, EMB], f32, kind="ExternalInput")
    idx_d = nc.dram_tensor("idx", [128, T_tot * 8], i16, kind="ExternalInput")
    lrw_d = nc.dram_tensor("lrw", [128, T_tot], f32, kind="ExternalInput")
    vls_d = nc.dram_tensor("vls", [128, T_tot], f32, kind="ExternalInput")
    noise_d = nc.dram_tensor("noise", [NL, RPC, EMB], f32, kind="ExternalInput")
    out_d = nc.dram_tensor("out", [RPC, EMB], f32, kind="ExternalOutput")
    ego_d = nc.dram_tensor("ego_slice", [RPC, EMB], f32)
    xg = [nc.dram_tensor(f"xg{i}", [N, EMB], f32, addr_space="Shared")
          for i in range(2)]

    # chunk lookup: tile index -> chunk (they are in increasing toff order)
    chunk_start_of = {}
    for ci, (c, toff, nt) in enumerate(chunks):
        chunk_start_of[toff] = ci

    with tile.TileContext(nc) as tc:
        nc.gpsimd.load_library(mlp)
        with tc.tile_pool(name="const", bufs=1) as constp, \
             tc.tile_pool(name="big", bufs=1) as big, \
             tc.tile_pool(name="gp", bufs=3) as gp, \
             tc.tile_pool(name="mp", bufs=3) as mp, \
             tc.tile_pool(name="hp", bufs=6) as hp, \
             tc.tile_pool(name="ep", bufs=2) as ep, \
             tc.tile_pool(name="pp", bufs=8, space="PSUM") as pp:

            iota_i = constp.tile([128, 128], i32)
            # bf16 iota: the H-build tensor_scalar then reads 16-bit packed
            # on ONE SBUF port (2x_1P) instead of fp32 2x_2P, which would
            # lock the DVE<->GpSimd shared port and stall SWDGE gather
            # descriptor generation.
            iota_f = constp.tile([128, 128], mybir.dt.bfloat16)
            nc.gpsimd.iota(iota_i[:], pattern=[[1, 128]], base=0,
                           channel_multiplier=0)
            nc.vector.tensor_copy(iota_f[:], iota_i[:])

            acc = big.tile([128, NRB, EMB], f32)
            spmm = big.tile([128, NRB, EMB], f32)
            nzb = big.tile([128, NRB, EMB], f32)
            n2 = big.tile([128, NRB], f32)
            nrm = big.tile([128, NRB], f32)
            rinv = big.tile([128, NRB], f32)

            nc.vector.memset(acc[:], 0.0)
            nc.vector.memset(nzb[:], 0.0)

            for layer in range(NL):
                src_t = [x0, xg[0], xg[1]][layer] if mode == "full" else x0
                nc.vector.memset(spmm[:], 0.0)

                # ---- noise prep (independent of spmm; overlaps gathers) ----
                nc.sync.dma_start(
                    nzb[:, :FULL_RB, :],
                    noise_d[layer, :FULL_RB * 128, :]
                    .rearrange("(b p) d -> p b d", p=128))
                nc.sync.dma_start(nzb[0:TAIL_R, FULL_RB, :],
                                  noise_d[layer, FULL_RB * 128:, :])
                for c0 in range(0, NRB, EP_C):
                    n = min(EP_C, NRB - c0)
                    sq = ep.tile([128, EP_C, EMB], f32, tag="sq")
                    nc.vector.tensor_tensor(
                        out=sq[:, :n, :], in0=nzb[:, c0:c0 + n, :],
                        in1=nzb[:, c0:c0 + n, :], op=Alu.mult)
                    nc.vector.tensor_reduce(
                        out=n2[:, c0:c0 + n], in_=sq[:, :n, :],
                        axis=mybir.AxisListType.X, op=Alu.add)
                nc.scalar.sqrt(nrm[:, :], n2[:, :])
                nc.vector.reciprocal(rinv[:, :], nrm[:, :])
                for b in range(NRB):
                    nc.vector.tensor_scalar(
                        out=nzb[:, b, :], in0=nzb[:, b, :],
                        scalar1=rinv[:, b:b + 1], scalar2=float(EPS),
                        op0=Alu.mult, op1=Alu.mult)

                def epilogue_chunk(c0, n):
                    # ego = spmm + sign(spmm)*nn ; acc += ego ; store slice
                    sg = ep.tile([128, EP_C, EMB], f32, tag="sg")
                    nc.scalar.sign(sg[:, :n, :], spmm[:, c0:c0 + n, :])
                    nc.vector.tensor_tensor(
                        out=sg[:, :n, :], in0=sg[:, :n, :],
                        in1=nzb[:, c0:c0 + n, :], op=Alu.mult)
                    nc.vector.tensor_tensor(
                        out=spmm[:, c0:c0 + n, :], in0=spmm[:, c0:c0 + n, :],
                        in1=sg[:, :n, :], op=Alu.add)
                    nc.vector.tensor_tensor(
                        out=acc[:, c0:c0 + n, :], in0=acc[:, c0:c0 + n, :],
                        in1=spmm[:, c0:c0 + n, :], op=Alu.add)
                    if layer < NL - 1:
                        nfull = min(c0 + n, FULL_RB) - c0
                        if nfull > 0:
                            nc.sync.dma_start(
                                ego_d[c0 * 128:(c0 + nfull) * 128, :]
                                .rearrange("(b p) d -> p b d", p=128),
                                spmm[:, c0:c0 + nfull, :])
                        if c0 + n > FULL_RB:
                            nc.sync.dma_start(
                                ego_d[FULL_RB * 128:, :],
                                spmm[0:TAIL_R, FULL_RB, :])

                cur = {"g": None, "lrow": None, "vals": None, "start": -1,
                       "nt": 0}
                if mode == "compute_only":
                    dg = gp.tile([128, CH_T, EMB], f32, tag="g")
                    dl = mp.tile([128, CH_T], f32, tag="lrow")
                    dv = mp.tile([128, CH_T], f32, tag="vals")
                    nc.vector.memset(dg[:], 0.25)
                    nc.vector.memset(dl[:], 1.0)
                    nc.vector.memset(dv[:], 0.5)
                    cur.update(g=dg, lrow=dl, vals=dv, start=0)

                def begin_chunk(ci):
                    cbv, toff, nt = chunks[ci]
                    qn = ci % 4
                    idxc = mp.tile([128, CH_T * 8], i16, tag="idx")
                    nc.sync.dma_start(idxc[:, :nt * 8],
                                      idx_d[:, toff * 8:(toff + nt) * 8])
                    lrowc = mp.tile([128, CH_T], f32, tag="lrow")
                    nc.sync.dma_start(lrowc[:, :nt],
                                      lrw_d[:, toff:toff + nt])
                    valsc = mp.tile([128, CH_T], f32, tag="vals")
                    nc.sync.dma_start(valsc[:, :nt],
                                      vls_d[:, toff:toff + nt])
                    g = gp.tile([128, CH_T, EMB], f32, tag="g")
                    nc.gpsimd.dma_gather(
                        g[:, :nt, :],
                        src_t[cbv * RPC:(cbv + 1) * RPC, :],
                        idxc[:, :nt * 8], nt * 128, nt * 128, EMB,
                        single_packet=False, queue_num=qn)
                    cur["g"], cur["lrow"], cur["vals"] = g, lrowc, valsc
                    cur["start"], cur["nt"] = toff, nt

                do_dma = mode in ("full", "gather_only", "nocc")
                do_compute = mode in ("full", "compute_only", "nocc")
                for c in range(NCB):
                    for b in range(NRB):
                        n_t = int(seg_tiles[c, b])
                        if n_t > 0:
                            g0 = int(seg_offsets[c, b])
                            if do_compute:
                                ps = pp.tile([128, EMB], f32, space="PSUM",
                                             tag="ps")
                            for j in range(n_t):
                                t = g0 + j
                                if t in chunk_start_of and do_dma:
                                    begin_chunk(chunk_start_of[t])
                                if not do_compute:
                                    continue
                                col = (t - cur["start"]) if do_dma else 0
                                h = hp.tile([128, 128], f32, tag="h")
                                nc.vector.tensor_scalar(
                                    out=h[:], in0=iota_f[:],
                                    scalar1=cur["lrow"][:, col:col + 1],
                                    scalar2=cur["vals"][:, col:col + 1],
                                    op0=Alu.is_equal, op1=Alu.mult)
                                nc.tensor.matmul(
                                    out=ps[:], lhsT=h[:],
                                    rhs=cur["g"][:, col, :],
                                    start=(j == 0), stop=(j == n_t - 1))
                            if do_compute:
                                nc.vector.tensor_tensor(
                                    out=spmm[:, b, :], in0=spmm[:, b, :],
                                    in1=ps[:], op=Alu.add)
                        if c == NCB - 1 and ((b + 1) % EP_C == 0
                                             or b == NRB - 1):
                            c0 = (b // EP_C) * EP_C
                            if do_compute:
                                epilogue_chunk(c0, b - c0 + 1)

                if layer < NL - 1 and mode == "full":
                    nc.gpsimd.collective_compute(
                        "AllGather", mybir.AluOpType.bypass,
                        replica_groups=[list(range(NCORES))],
                        ins=[ego_d[:]], outs=[xg[layer][:]])

            nc.vector.tensor_scalar_mul(acc[:], acc[:], 1.0 / NL)
            nc.sync.dma_start(
                out_d[:FULL_RB * 128, :].rearrange("(b p) d -> p b d", p=128),
                acc[:, :FULL_RB, :])
            nc.sync.dma_start(out_d[FULL_RB * 128:, :],
                              acc[0:TAIL_R, FULL_RB, :])
    nc.compile()
    return nc


def _run(inputs, trace=False):
    from concourse.bass_utils import run_bass_kernel_spmd

    user_emb = np.asarray(inputs["user_emb"], dtype=np.float32)
    item_emb = np.asarray(inputs["item_emb"], dtype=np.float32)
    noise = np.asarray(inputs["noise"], dtype=np.float32)
    x0 = np.concatenate([user_emb, item_emb], axis=0)

    seg_tiles, seg_offsets, chunks, T_tot, streams = _preprocess(
        inputs["adj_rows"], inputs["adj_cols"], inputs["adj_vals"])

    nc = _build_program(seg_tiles, seg_offsets, chunks, T_tot)

    in_maps = []
    for k in range(NCORES):
        in_maps.append({
            "x0": x0,
            "idx": streams[k]["idx"],
            "lrw": streams[k]["lrow"],
            "vls": streams[k]["vals"],
            "noise": np.ascontiguousarray(noise[:, k * RPC:(k + 1) * RPC, :]),
        })
    res = run_bass_kernel_spmd(nc, in_maps, core_ids=list(range(NCORES)),
                               trace=trace)
    res._timing_ctx = (nc, in_maps)
    acc = np.concatenate([res.results[k]["out"] for k in range(NCORES)],
                         axis=0)
    user_all = acc[:USER_NUM]
    item_all = acc[USER_NUM:]
    outs = (user_all, item_all,
            np.asarray(inputs["user_prototypes"], dtype=np.float32),
            np.asarray(inputs["item_prototypes"], dtype=np.float32))
    return outs, res


def _time_neff(nc, in_maps, reps=5):
    """Wall-clock the NEFF execution with device-resident inputs.

    Mirrors bass2jax.run_bass_via_pjrt's multi-core path but without
    donation so the same device buffers can be re-executed."""
    import time

    import jax
    import numpy as np_
    from jax.sharding import Mesh, NamedSharding, PartitionSpec
    from jax.experimental.shard_map import shard_map

    import concourse.mybir as mybir
    from concourse import bass2jax

    bass2jax.install_neuronx_cc_hook()

    partition_name = (nc.partition_id_tensor.name
                      if nc.partition_id_tensor else None)
    in_names, out_names, out_avals, zero_outs = [], [], [], []
    for alloc in nc.m.functions[0].allocations:
        if not isinstance(alloc, mybir.MemoryLocationSet):
            continue
        name = alloc.memorylocations[0].name
        if alloc.kind == "ExternalInput":
            if name != partition_name:
                in_names.append(name)
        elif alloc.kind == "ExternalOutput":
            shape = tuple(alloc.tensor_shape)
            dtype = mybir.dt.np(alloc.dtype)
            out_names.append(name)
            out_avals.append(jax.core.ShapedArray(shape, dtype))
            zero_outs.append(np_.zeros(shape, dtype))
    n_params = len(in_names)
    all_names = in_names + out_names

    bind_names = list(all_names)
    if partition_name is not None:
        bind_names.append(partition_name)

    def _body(*args):
        operands = list(args)
        if partition_name is not None:
            operands.append(bass2jax.partition_id_tensor())
        outs = bass2jax._bass_exec_p.bind(
            *operands,
            out_avals=tuple(out_avals),
            in_names=tuple(bind_names),
            out_names=tuple(out_names),
            lowering_input_output_aliases=(),
            sim_require_finite=True,
            sim_require_nnan=True,
            nc=nc,
        )
        return tuple(outs)

    devices = jax.devices()[:NCORES]
    mesh = Mesh(np_.asarray(devices), ("core",))
    nspec = len(all_names)
    sharded = jax.jit(shard_map(
        _body, mesh=mesh, in_specs=(PartitionSpec("core"),) * nspec,
        out_specs=(PartitionSpec("core"),) * len(out_names), check_rep=False))

    sh = NamedSharding(mesh, PartitionSpec("core"))
    dev_args = []
    for i, name in enumerate(all_names):
        if i < n_params:
            arr = np_.concatenate(
                [np_.asarray(m[name]) for m in in_maps], axis=0)
        else:
            z = zero_outs[i - n_params]
            arr = np_.zeros((NCORES * z.shape[0], *z.shape[1:]), z.dtype)
        dev_args.append(jax.device_put(arr, sh))

    times = []
    for _ in range(reps):
        t0 = time.perf_counter()
        out = sharded(*dev_args)
        jax.block_until_ready(out)
        times.append(time.perf_counter() - t0)
    return times


def kernel(**inputs):
    outs, _ = _run(inputs, trace=False)
    return outs


# revision 22
# speedup vs baseline: 1.6159x; 1.2447x over previous
"""DDAU encoder (3-layer noisy GNN message passing) on 8 trn2 NeuronCores.

Strategy (1D row sharding):
  - Core k owns output rows [k*18750, (k+1)*18750).
  - Host sorts each core's edges by (source col-block, dest row-block) and
    pads each (col-block, row-block) segment to a multiple of 128 edges with
    a cross-core-uniform tile count, so one SPMD program serves all cores.
  - Per 128-edge tile: dma_gather pulls x[col] rows from HBM (edge on
    partition), a fused DVE tensor_scalar builds H[e,m] = (m==lrow[e])*val[e],
    and the tensor engine accumulates H^T @ G into the 128-row output block
    in PSUM. No scatter DMA anywhere.
  - Per layer: noise injection epilogue on the owned slice, then an HBM
    AllGather shares each core's updated slice for the next layer's gathers.
"""

import numpy as np

N = 150000
USER_NUM = 100000
NCORES = 8
RPC = N // NCORES          # 18750 rows per core
EMB = 64
NRB = (RPC + 127) // 128   # 147 row blocks; last block has 62 rows
FULL_RB = RPC // 128       # 146 full blocks
TAIL_R = RPC - FULL_RB * 128  # 62
NCB = NCORES               # 8 col blocks of RPC rows each (idx fits int16)
NL = 3
EPS = 0.1
CH_T = 32                  # gather chunk size in tiles (32*128 idxs/call)
EP_C = 21                  # epilogue row-block chunk (7 chunks of 21)


def _preprocess(adj_rows, adj_cols, adj_vals):
    rows = np.asarray(adj_rows).astype(np.int64)
    cols = np.asarray(adj_cols).astype(np.int64)
    vals = np.asarray(adj_vals).astype(np.float32)

    core = rows // RPC
    lr = rows - core * RPC
    rb = lr >> 7
    lrow128 = (lr & 127).astype(np.float32)
    cbv = cols // RPC
    lcol = (cols - cbv * RPC).astype(np.int16)

    per_core = []
    counts = np.zeros((NCORES, NCB, NRB), np.int64)
    for k in range(NCORES):
        m = core == k
        key = (cbv[m] * NRB + rb[m]).astype(np.int64)
        order = np.argsort(key, kind="stable")
        per_core.append((lcol[m][order], lrow128[m][order], vals[m][order]))
        counts[k] = np.bincount(key, minlength=NCB * NRB).reshape(NCB, NRB)

    seg_tiles = -(-counts.max(axis=0) // 128)          # [NCB, NRB]
    T_tot = int(seg_tiles.sum())

    seg_offsets = np.zeros((NCB, NRB), np.int64)
    toff = 0
    for c in range(NCB):
        for b in range(NRB):
            seg_offsets[c, b] = toff
            toff += int(seg_tiles[c, b])

    chunks = []
    for c in range(NCB):
        start = int(seg_offsets[c, 0])
        end = int(seg_offsets[c + 1, 0]) if c + 1 < NCB else T_tot
        t = start
        while t < end:
            n = min(CH_T, end - t)
            chunks.append((c, t, n))
            t += n

    streams = []
    for k in range(NCORES):
        lc, lrw, vl = per_core[k]
        E_pad = T_tot * 128
        lcol_s = np.zeros(E_pad, np.int16)
        lrow_s = np.full(E_pad, -1.0, np.float32)
        vals_s = np.zeros(E_pad, np.float32)
        src_ofs = np.zeros(NCB * NRB + 1, np.int64)
        np.cumsum(counts[k].reshape(-1), out=src_ofs[1:])
        for c in range(NCB):
            for b in range(NRB):
                s = int(src_ofs[c * NRB + b])
                e = int(src_ofs[c * NRB + b + 1])
                d = int(seg_offsets[c, b]) * 128
                lcol_s[d:d + e - s] = lc[s:e]
                lrow_s[d:d + e - s] = lrw[s:e]
                vals_s[d:d + e - s] = vl[s:e]
        streams.append({
            "idx": np.tile(lcol_s.reshape(-1, 16).T, (8, 1)),        # [128,T*8]
            "lrow": np.ascontiguousarray(lrow_s.reshape(-1, 128).T),  # [128,T]
            "vals": np.ascontiguousarray(vals_s.reshape(-1, 128).T),
        })
    return seg_tiles, seg_offsets, chunks, T_tot, streams


def _build_program(seg_tiles, seg_offsets, chunks, T_tot, mode="full"):
    import concourse.bacc as bacc
    import concourse.mybir as mybir
    import concourse.tile as tile
    from concourse.library_config import mlp

    f32 = mybir.dt.float32
    i16 = mybir.dt.int16
    i32 = mybir.dt.int32
    Alu = mybir.AluOpType

    nc = bacc.Bacc("TRN2", target_bir_lowering=False, debug=False,
                   num_devices=NCORES, num_swdge_queues=4)
    x0 = nc.dram_tensor("x0", [N, EMB], f32, kind="ExternalInput")
    idx_d = nc.dram_tensor("idx", [128, T_tot * 8], i16, kind="ExternalInput")
    lrw_d = nc.dram_tensor("lrw", [128, T_tot], f32, kind="ExternalInput")
    vls_d = nc.dram_tensor("vls", [128, T_tot], f32, kind="ExternalInput")
    noise_d = nc.dram_tensor("noise", [NL, RPC, EMB], f32, kind="ExternalInput")
    out_d = nc.dram_tensor("out", [RPC, EMB], f32, kind="ExternalOutput")
    ego_d = nc.dram_tensor("ego_slice", [RPC, EMB], f32)
    xg = [nc.dram_tensor(f"xg{i}", [N, EMB], f32, addr_space="Shared")
          for i in range(2)]

    # chunk lookup: tile index -> chunk (they are in increasing toff order)
    chunk_start_of = {}
    for ci, (c, toff, nt) in enumerate(chunks):
        chunk_start_of[toff] = ci

    with tile.TileContext(nc) as tc:
        nc.gpsimd.load_library(mlp)
        with tc.tile_pool(name="const", bufs=1) as constp, \
             tc.tile_pool(name="big", bufs=1) as big, \
             tc.tile_pool(name="gp", bufs=6) as gp, \
             tc.tile_pool(name="mp", bufs=6) as mp, \
             tc.tile_pool(name="hp", bufs=6) as hp, \
             tc.tile_pool(name="ep", bufs=2) as ep, \
             tc.tile_pool(name="pp", bufs=8, space="PSUM") as pp:

            iota_i = constp.tile([128, 128], i32)
            # bf16 iota: the H-build tensor_scalar then reads 16-bit packed
            # on ONE SBUF port (2x_1P) instead of fp32 2x_2P, which would
            # lock the DVE<->GpSimd shared port and stall SWDGE gather
            # descriptor generation.
            iota_f = constp.tile([128, 128], mybir.dt.bfloat16)
            nc.gpsimd.iota(iota_i[:], pattern=[[1, 128]], base=0,
                           channel_multiplier=0)
            nc.vector.tensor_copy(iota_f[:], iota_i[:])

            acc = big.tile([128, NRB, EMB], f32)
            spmm = big.tile([128, NRB, EMB], f32)
            nzb = big.tile([128, NRB, EMB], f32)
            n2 = big.tile([128, NRB], f32)
            nrm = big.tile([128, NRB], f32)
            rinv = big.tile([128, NRB], f32)

            nc.vector.memset(acc[:], 0.0)
            nc.vector.memset(nzb[:], 0.0)

            for layer in range(NL):
                src_t = [x0, xg[0], xg[1]][layer] if mode == "full" else x0
                nc.vector.memset(spmm[:], 0.0)

                # ---- noise prep (independent of spmm; overlaps gathers) ----
                nc.sync.dma_start(
                    nzb[:, :FULL_RB, :],
                    noise_d[layer, :FULL_RB * 128, :]
                    .rearrange("(b p) d -> p b d", p=128))
                nc.sync.dma_start(nzb[0:TAIL_R, FULL_RB, :],
                                  noise_d[layer, FULL_RB * 128:, :])
                for c0 in range(0, NRB, EP_C):
                    n = min(EP_C, NRB - c0)
                    sq = ep.tile([128, EP_C, EMB], f32, tag="sq")
                    nc.vector.tensor_tensor(
                        out=sq[:, :n, :], in0=nzb[:, c0:c0 + n, :],
                        in1=nzb[:, c0:c0 + n, :], op=Alu.mult)
                    nc.vector.tensor_reduce(
                        out=n2[:, c0:c0 + n], in_=sq[:, :n, :],
                        axis=mybir.AxisListType.X, op=Alu.add)
                nc.scalar.sqrt(nrm[:, :], n2[:, :])
                nc.vector.reciprocal(rinv[:, :], nrm[:, :])
                for b in range(NRB):
                    nc.vector.tensor_scalar(
                        out=nzb[:, b, :], in0=nzb[:, b, :],
                        scalar1=rinv[:, b:b + 1], scalar2=float(EPS),
                        op0=Alu.mult, op1=Alu.mult)

                def epilogue_chunk(c0, n):
                    # ego = spmm + sign(spmm)*nn ; acc += ego ; store slice
                    sg = ep.tile([128, EP_C, EMB], f32, tag="sg")
                    nc.scalar.sign(sg[:, :n, :], spmm[:, c0:c0 + n, :])
                    nc.vector.tensor_tensor(
                        out=sg[:, :n, :], in0=sg[:, :n, :],
                        in1=nzb[:, c0:c0 + n, :], op=Alu.mult)
                    nc.vector.tensor_tensor(
                        out=spmm[:, c0:c0 + n, :], in0=spmm[:, c0:c0 + n, :],
                        in1=sg[:, :n, :], op=Alu.add)
                    nc.vector.tensor_tensor(
                        out=acc[:, c0:c0 + n, :], in0=acc[:, c0:c0 + n, :],
                        in1=spmm[:, c0:c0 + n, :], op=Alu.add)
                    if layer < NL - 1:
                        nfull = min(c0 + n, FULL_RB) - c0
                        if nfull > 0:
                            nc.sync.dma_start(
                                ego_d[c0 * 128:(c0 + nfull) * 128, :]
                                .rearrange("(b p) d -> p b d", p=128),
                                spmm[:, c0:c0 + nfull, :])
                        if c0 + n > FULL_RB:
                            nc.sync.dma_start(
                                ego_d[FULL_RB * 128:, :],
                                spmm[0:TAIL_R, FULL_RB, :])

                cur = {"g": None, "lrow": None, "vals": None, "start": -1,
                       "nt": 0}
                if mode == "compute_only":
                    dg = gp.tile([128, CH_T, EMB], f32, tag="g")
                    dl = mp.tile([128, CH_T], f32, tag="lrow")
                    dv = mp.tile([128, CH_T], f32, tag="vals")
                    nc.vector.memset(dg[:], 0.25)
                    nc.vector.memset(dl[:], 1.0)
                    nc.vector.memset(dv[:], 0.5)
                    cur.update(g=dg, lrow=dl, vals=dv, start=0)

                def begin_chunk(ci):
                    cbv, toff, nt = chunks[ci]
                    qn = ci % 4
                    idxc = mp.tile([128, CH_T * 8], i16, tag="idx")
                    nc.sync.dma_start(idxc[:, :nt * 8],
                                      idx_d[:, toff * 8:(toff + nt) * 8])
                    lrowc = mp.tile([128, CH_T], f32, tag="lrow")
                    nc.sync.dma_start(lrowc[:, :nt],
                                      lrw_d[:, toff:toff + nt])
                    valsc = mp.tile([128, CH_T], f32, tag="vals")
                    nc.sync.dma_start(valsc[:, :nt],
                                      vls_d[:, toff:toff + nt])
                    g = gp.tile([128, CH_T, EMB], f32, tag="g")
                    nc.gpsimd.dma_gather(
                        g[:, :nt, :],
                        src_t[cbv * RPC:(cbv + 1) * RPC, :],
                        idxc[:, :nt * 8], nt * 128, nt * 128, EMB,
                        single_packet=False, queue_num=qn)
                    cur["g"], cur["lrow"], cur["vals"] = g, lrowc, valsc
                    cur["start"], cur["nt"] = toff, nt

                do_dma = mode in ("full", "gather_only", "nocc")
                do_compute = mode in ("full", "compute_only", "nocc")
                for c in range(NCB):
                    for b in range(NRB):
                        n_t = int(seg_tiles[c, b])
                        if n_t > 0:
                            g0 = int(seg_offsets[c, b])
                            if do_compute:
                                ps = pp.tile([128, EMB], f32, space="PSUM",
                                             tag="ps")
                            for j in range(n_t):
                                t = g0 + j
                                if t in chunk_start_of and do_dma:
                                    begin_chunk(chunk_start_of[t])
                                if not do_compute:
                                    continue
                                col = (t - cur["start"]) if do_dma else 0
                                h = hp.tile([128, 128], f32, tag="h")
                                nc.vector.tensor_scalar(
                                    out=h[:], in0=iota_f[:],
                                    scalar1=cur["lrow"][:, col:col + 1],
                                    scalar2=cur["vals"][:, col:col + 1],
                                    op0=Alu.is_equal, op1=Alu.mult)
                                nc.tensor.matmul(
                                    out=ps[:], lhsT=h[:],
                                    rhs=cur["g"][:, col, :],
                                    start=(j == 0), stop=(j == n_t - 1))
                            if do_compute:
                                nc.vector.tensor_tensor(
                                    out=spmm[:, b, :], in0=spmm[:, b, :],
                                    in1=ps[:], op=Alu.add)
                        if c == NCB - 1 and ((b + 1) % EP_C == 0
                                             or b == NRB - 1):
                            c0 = (b // EP_C) * EP_C
                            if do_compute:
                                epilogue_chunk(c0, b - c0 + 1)

                if layer < NL - 1 and mode == "full":
                    nc.gpsimd.collective_compute(
                        "AllGather", mybir.AluOpType.bypass,
                        replica_groups=[list(range(NCORES))],
                        ins=[ego_d[:]], outs=[xg[layer][:]])

            nc.vector.tensor_scalar_mul(acc[:], acc[:], 1.0 / NL)
            nc.sync.dma_start(
                out_d[:FULL_RB * 128, :].rearrange("(b p) d -> p b d", p=128),
                acc[:, :FULL_RB, :])
            nc.sync.dma_start(out_d[FULL_RB * 128:, :],
                              acc[0:TAIL_R, FULL_RB, :])
    nc.compile()
    return nc


def _run(inputs, trace=False):
    from concourse.bass_utils import run_bass_kernel_spmd

    user_emb = np.asarray(inputs["user_emb"], dtype=np.float32)
    item_emb = np.asarray(inputs["item_emb"], dtype=np.float32)
    noise = np.asarray(inputs["noise"], dtype=np.float32)
    x0 = np.concatenate([user_emb, item_emb], axis=0)

    seg_tiles, seg_offsets, chunks, T_tot, streams = _preprocess(
        inputs["adj_rows"], inputs["adj_cols"], inputs["adj_vals"])

    nc = _build_program(seg_tiles, seg_offsets, chunks, T_tot)

    in_maps = []
    for k in range(NCORES):
        in_maps.append({
            "x0": x0,
            "idx": streams[k]["idx"],
            "lrw": streams[k]["lrow"],
            "vls": streams[k]["vals"],
            "noise": np.ascontiguousarray(noise[:, k * RPC:(k + 1) * RPC, :]),
        })
    res = run_bass_kernel_spmd(nc, in_maps, core_ids=list(range(NCORES)),
                               trace=trace)
    res._timing_ctx = (nc, in_maps)
    acc = np.concatenate([res.results[k]["out"] for k in range(NCORES)],
                         axis=0)
    user_all = acc[:USER_NUM]
    item_all = acc[USER_NUM:]
    outs = (user_all, item_all,
            np.asarray(inputs["user_prototypes"], dtype=np.float32),
            np.asarray(inputs["item_prototypes"], dtype=np.float32))
    return outs, res


def _time_neff(nc, in_maps, reps=5):
    """Wall-clock the NEFF execution with device-resident inputs.

    Mirrors bass2jax.run_bass_via_pjrt's multi-core path but without
    donation so the same device buffers can be re-executed."""
    import time

    import jax
    import numpy as np_
    from jax.sharding import Mesh, NamedSharding, PartitionSpec
    from jax.experimental.shard_map import shard_map

    import concourse.mybir as mybir
    from concourse import bass2jax

    bass2jax.install_neuronx_cc_hook()

    partition_name = (nc.partition_id_tensor.name
                      if nc.partition_id_tensor else None)
    in_names, out_names, out_avals, zero_outs = [], [], [], []
    for alloc in nc.m.functions[0].allocations:
        if not isinstance(alloc, mybir.MemoryLocationSet):
            continue
        name = alloc.memorylocations[0].name
        if alloc.kind == "ExternalInput":
            if name != partition_name:
                in_names.append(name)
        elif alloc.kind == "ExternalOutput":
            shape = tuple(alloc.tensor_shape)
            dtype = mybir.dt.np(alloc.dtype)
            out_names.append(name)
            out_avals.append(jax.core.ShapedArray(shape, dtype))
            zero_outs.append(np_.zeros(shape, dtype))
    n_params = len(in_names)
    all_names = in_names + out_names

    bind_names = list(all_names)
    if partition_name is not None:
        bind_names.append(partition_name)

    def _body(*args):
        operands = list(args)
        if partition_name is not None:
            operands.append(bass2jax.partition_id_tensor())
        outs = bass2jax._bass_exec_p.bind(
            *operands,
            out_avals=tuple(out_avals),
            in_names=tuple(bind_names),
            out_names=tuple(out_names),
            lowering_input_output_aliases=(),
            sim_require_finite=True,
            sim_require_nnan=True,
            nc=nc,
        )
        return tuple(outs)

    devices = jax.devices()[:NCORES]
    mesh = Mesh(np_.asarray(devices), ("core",))
    nspec = len(all_names)
    sharded = jax.jit(shard_map(
        _body, mesh=mesh, in_specs=(PartitionSpec("core"),) * nspec,
        out_specs=(PartitionSpec("core"),) * len(out_names), check_rep=False))

    sh = NamedSharding(mesh, PartitionSpec("core"))
    dev_args = []
    for i, name in enumerate(all_names):
        if i < n_params:
            arr = np_.concatenate(
                [np_.asarray(m[name]) for m in in_maps], axis=0)
        else:
            z = zero_outs[i - n_params]
            arr = np_.zeros((NCORES * z.shape[0], *z.shape[1:]), z.dtype)
        dev_args.append(jax.device_put(arr, sh))

    times = []
    for _ in range(reps):
        t0 = time.perf_counter()
        out = sharded(*dev_args)
        jax.block_until_ready(out)
        times.append(time.perf_counter() - t0)
    return times


def kernel(**inputs):
    outs, _ = _run(inputs, trace=False)
    return outs


# revision 23
# speedup vs baseline: 1.7264x; 1.0684x over previous
"""DDAU encoder (3-layer noisy GNN message passing) on 8 trn2 NeuronCores.

Strategy (1D row sharding):
  - Core k owns output rows [k*18750, (k+1)*18750).
  - Host sorts each core's edges by (source col-block, dest row-block) and
    pads each (col-block, row-block) segment to a multiple of 128 edges with
    a cross-core-uniform tile count, so one SPMD program serves all cores.
  - Per 128-edge tile: dma_gather pulls x[col] rows from HBM (edge on
    partition), a fused DVE tensor_scalar builds H[e,m] = (m==lrow[e])*val[e],
    and the tensor engine accumulates H^T @ G into the 128-row output block
    in PSUM. No scatter DMA anywhere.
  - Per layer: noise injection epilogue on the owned slice, then an HBM
    AllGather shares each core's updated slice for the next layer's gathers.
"""

import numpy as np

N = 150000
USER_NUM = 100000
NCORES = 8
RPC = N // NCORES          # 18750 rows per core
EMB = 64
NRB = (RPC + 127) // 128   # 147 row blocks; last block has 62 rows
FULL_RB = RPC // 128       # 146 full blocks
TAIL_R = RPC - FULL_RB * 128  # 62
NCB = NCORES               # 8 col blocks of RPC rows each (idx fits int16)
NL = 3
EPS = 0.1
CH_T = 16                  # gather chunk size in tiles (16*128 idxs/call)
EP_C = 21                  # epilogue row-block chunk (7 chunks of 21)


def _preprocess(adj_rows, adj_cols, adj_vals):
    rows = np.asarray(adj_rows).astype(np.int64)
    cols = np.asarray(adj_cols).astype(np.int64)
    vals = np.asarray(adj_vals).astype(np.float32)

    core = rows // RPC
    lr = rows - core * RPC
    rb = lr >> 7
    lrow128 = (lr & 127).astype(np.float32)
    cbv = cols // RPC
    lcol = (cols - cbv * RPC).astype(np.int16)

    per_core = []
    counts = np.zeros((NCORES, NCB, NRB), np.int64)
    for k in range(NCORES):
        m = core == k
        key = (cbv[m] * NRB + rb[m]).astype(np.int64)
        order = np.argsort(key, kind="stable")
        per_core.append((lcol[m][order], lrow128[m][order], vals[m][order]))
        counts[k] = np.bincount(key, minlength=NCB * NRB).reshape(NCB, NRB)

    seg_tiles = -(-counts.max(axis=0) // 128)          # [NCB, NRB]
    T_tot = int(seg_tiles.sum())

    seg_offsets = np.zeros((NCB, NRB), np.int64)
    toff = 0
    for c in range(NCB):
        for b in range(NRB):
            seg_offsets[c, b] = toff
            toff += int(seg_tiles[c, b])

    chunks = []
    for c in range(NCB):
        start = int(seg_offsets[c, 0])
        end = int(seg_offsets[c + 1, 0]) if c + 1 < NCB else T_tot
        t = start
        while t < end:
            n = min(CH_T, end - t)
            chunks.append((c, t, n))
            t += n

    streams = []
    for k in range(NCORES):
        lc, lrw, vl = per_core[k]
        E_pad = T_tot * 128
        lcol_s = np.zeros(E_pad, np.int16)
        lrow_s = np.full(E_pad, -1.0, np.float32)
        vals_s = np.zeros(E_pad, np.float32)
        src_ofs = np.zeros(NCB * NRB + 1, np.int64)
        np.cumsum(counts[k].reshape(-1), out=src_ofs[1:])
        for c in range(NCB):
            for b in range(NRB):
                s = int(src_ofs[c * NRB + b])
                e = int(src_ofs[c * NRB + b + 1])
                d = int(seg_offsets[c, b]) * 128
                lcol_s[d:d + e - s] = lc[s:e]
                lrow_s[d:d + e - s] = lrw[s:e]
                vals_s[d:d + e - s] = vl[s:e]
        streams.append({
            "idx": np.tile(lcol_s.reshape(-1, 16).T, (8, 1)),        # [128,T*8]
            "lrow": np.ascontiguousarray(lrow_s.reshape(-1, 128).T),  # [128,T]
            "vals": np.ascontiguousarray(vals_s.reshape(-1, 128).T),
        })
    return seg_tiles, seg_offsets, chunks, T_tot, streams


def _build_program(seg_tiles, seg_offsets, chunks, T_tot, mode="full"):
    import concourse.bacc as bacc
    import concourse.mybir as mybir
    import concourse.tile as tile
    from concourse.library_config import mlp

    f32 = mybir.dt.float32
    i16 = mybir.dt.int16
    i32 = mybir.dt.int32
    Alu = mybir.AluOpType

    nc = bacc.Bacc("TRN2", target_bir_lowering=False, debug=False,
                   num_devices=NCORES, num_swdge_queues=4)
    x0 = nc.dram_tensor("x0", [N, EMB], f32, kind="ExternalInput")
    idx_d = nc.dram_tensor("idx", [128, T_tot * 8], i16, kind="ExternalInput")
    lrw_d = nc.dram_tensor("lrw", [128, T_tot], f32, kind="ExternalInput")
    vls_d = nc.dram_tensor("vls", [128, T_tot], f32, kind="ExternalInput")
    noise_d = nc.dram_tensor("noise", [NL, RPC, EMB], f32, kind="ExternalInput")
    out_d = nc.dram_tensor("out", [RPC, EMB], f32, kind="ExternalOutput")
    ego_d = nc.dram_tensor("ego_slice", [RPC, EMB], f32)
    xg = [nc.dram_tensor(f"xg{i}", [N, EMB], f32, addr_space="Shared")
          for i in range(2)]

    # chunk lookup: tile index -> chunk (they are in increasing toff order)
    chunk_start_of = {}
    for ci, (c, toff, nt) in enumerate(chunks):
        chunk_start_of[toff] = ci

    with tile.TileContext(nc) as tc:
        nc.gpsimd.load_library(mlp)
        with tc.tile_pool(name="const", bufs=1) as constp, \
             tc.tile_pool(name="big", bufs=1) as big, \
             tc.tile_pool(name="gp", bufs=12) as gp, \
             tc.tile_pool(name="mp", bufs=12) as mp, \
             tc.tile_pool(name="hp", bufs=6) as hp, \
             tc.tile_pool(name="ep", bufs=2) as ep, \
             tc.tile_pool(name="pp", bufs=8, space="PSUM") as pp:

            iota_i = constp.tile([128, 128], i32)
            # bf16 iota: the H-build tensor_scalar then reads 16-bit packed
            # on ONE SBUF port (2x_1P) instead of fp32 2x_2P, which would
            # lock the DVE<->GpSimd shared port and stall SWDGE gather
            # descriptor generation.
            iota_f = constp.tile([128, 128], mybir.dt.bfloat16)
            nc.gpsimd.iota(iota_i[:], pattern=[[1, 128]], base=0,
                           channel_multiplier=0)
            nc.vector.tensor_copy(iota_f[:], iota_i[:])

            acc = big.tile([128, NRB, EMB], f32)
            spmm = big.tile([128, NRB, EMB], f32)
            nzb = big.tile([128, NRB, EMB], f32)
            n2 = big.tile([128, NRB], f32)
            nrm = big.tile([128, NRB], f32)
            rinv = big.tile([128, NRB], f32)

            nc.vector.memset(acc[:], 0.0)
            nc.vector.memset(nzb[:], 0.0)

            for layer in range(NL):
                src_t = [x0, xg[0], xg[1]][layer] if mode == "full" else x0
                nc.vector.memset(spmm[:], 0.0)

                # ---- noise prep (independent of spmm; overlaps gathers) ----
                nc.sync.dma_start(
                    nzb[:, :FULL_RB, :],
                    noise_d[layer, :FULL_RB * 128, :]
                    .rearrange("(b p) d -> p b d", p=128))
                nc.sync.dma_start(nzb[0:TAIL_R, FULL_RB, :],
                                  noise_d[layer, FULL_RB * 128:, :])
                for c0 in range(0, NRB, EP_C):
                    n = min(EP_C, NRB - c0)
                    sq = ep.tile([128, EP_C, EMB], f32, tag="sq")
                    nc.vector.tensor_tensor(
                        out=sq[:, :n, :], in0=nzb[:, c0:c0 + n, :],
                        in1=nzb[:, c0:c0 + n, :], op=Alu.mult)
                    nc.vector.tensor_reduce(
                        out=n2[:, c0:c0 + n], in_=sq[:, :n, :],
                        axis=mybir.AxisListType.X, op=Alu.add)
                nc.scalar.sqrt(nrm[:, :], n2[:, :])
                nc.vector.reciprocal(rinv[:, :], nrm[:, :])
                for b in range(NRB):
                    nc.vector.tensor_scalar(
                        out=nzb[:, b, :], in0=nzb[:, b, :],
                        scalar1=rinv[:, b:b + 1], scalar2=float(EPS),
                        op0=Alu.mult, op1=Alu.mult)

                def epilogue_chunk(c0, n):
                    # ego = spmm + sign(spmm)*nn ; acc += ego ; store slice
                    sg = ep.tile([128, EP_C, EMB], f32, tag="sg")
                    nc.scalar.sign(sg[:, :n, :], spmm[:, c0:c0 + n, :])
                    nc.vector.tensor_tensor(
                        out=sg[:, :n, :], in0=sg[:, :n, :],
                        in1=nzb[:, c0:c0 + n, :], op=Alu.mult)
                    nc.vector.tensor_tensor(
                        out=spmm[:, c0:c0 + n, :], in0=spmm[:, c0:c0 + n, :],
                        in1=sg[:, :n, :], op=Alu.add)
                    nc.vector.tensor_tensor(
                        out=acc[:, c0:c0 + n, :], in0=acc[:, c0:c0 + n, :],
                        in1=spmm[:, c0:c0 + n, :], op=Alu.add)
                    if layer < NL - 1:
                        nfull = min(c0 + n, FULL_RB) - c0
                        if nfull > 0:
                            nc.sync.dma_start(
                                ego_d[c0 * 128:(c0 + nfull) * 128, :]
                                .rearrange("(b p) d -> p b d", p=128),
                                spmm[:, c0:c0 + nfull, :])
                        if c0 + n > FULL_RB:
                            nc.sync.dma_start(
                                ego_d[FULL_RB * 128:, :],
                                spmm[0:TAIL_R, FULL_RB, :])

                cur = {"g": None, "lrow": None, "vals": None, "start": -1,
                       "nt": 0}
                if mode == "compute_only":
                    dg = gp.tile([128, CH_T, EMB], f32, tag="g")
                    dl = mp.tile([128, CH_T], f32, tag="lrow")
                    dv = mp.tile([128, CH_T], f32, tag="vals")
                    nc.vector.memset(dg[:], 0.25)
                    nc.vector.memset(dl[:], 1.0)
                    nc.vector.memset(dv[:], 0.5)
                    cur.update(g=dg, lrow=dl, vals=dv, start=0)

                def begin_chunk(ci):
                    cbv, toff, nt = chunks[ci]
                    qn = ci % 4
                    idxc = mp.tile([128, CH_T * 8], i16, tag="idx")
                    nc.sync.dma_start(idxc[:, :nt * 8],
                                      idx_d[:, toff * 8:(toff + nt) * 8])
                    lrowc = mp.tile([128, CH_T], f32, tag="lrow")
                    nc.sync.dma_start(lrowc[:, :nt],
                                      lrw_d[:, toff:toff + nt])
                    valsc = mp.tile([128, CH_T], f32, tag="vals")
                    nc.sync.dma_start(valsc[:, :nt],
                                      vls_d[:, toff:toff + nt])
                    g = gp.tile([128, CH_T, EMB], f32, tag="g")
                    nc.gpsimd.dma_gather(
                        g[:, :nt, :],
                        src_t[cbv * RPC:(cbv + 1) * RPC, :],
                        idxc[:, :nt * 8], nt * 128, nt * 128, EMB,
                        single_packet=False, queue_num=qn)
                    cur["g"], cur["lrow"], cur["vals"] = g, lrowc, valsc
                    cur["start"], cur["nt"] = toff, nt

                do_dma = mode in ("full", "gather_only", "nocc")
                do_compute = mode in ("full", "compute_only", "nocc")
                for c in range(NCB):
                    for b in range(NRB):
                        n_t = int(seg_tiles[c, b])
                        if n_t > 0:
                            g0 = int(seg_offsets[c, b])
                            if do_compute:
                                ps = pp.tile([128, EMB], f32, space="PSUM",
                                             tag="ps")
                            for j in range(n_t):
                                t = g0 + j
                                if t in chunk_start_of and do_dma:
                                    begin_chunk(chunk_start_of[t])
                                if not do_compute:
                                    continue
                                col = (t - cur["start"]) if do_dma else 0
                                h = hp.tile([128, 128], f32, tag="h")
                                nc.vector.tensor_scalar(
                                    out=h[:], in0=iota_f[:],
                                    scalar1=cur["lrow"][:, col:col + 1],
                                    scalar2=cur["vals"][:, col:col + 1],
                                    op0=Alu.is_equal, op1=Alu.mult)
                                nc.tensor.matmul(
                                    out=ps[:], lhsT=h[:],
                                    rhs=cur["g"][:, col, :],
                                    start=(j == 0), stop=(j == n_t - 1))
                            if do_compute:
                                nc.vector.tensor_tensor(
                                    out=spmm[:, b, :], in0=spmm[:, b, :],
                                    in1=ps[:], op=Alu.add)
                        if c == NCB - 1 and ((b + 1) % EP_C == 0
                                             or b == NRB - 1):
                            c0 = (b // EP_C) * EP_C
                            if do_compute:
                                epilogue_chunk(c0, b - c0 + 1)

                if layer < NL - 1 and mode == "full":
                    nc.gpsimd.collective_compute(
                        "AllGather", mybir.AluOpType.bypass,
                        replica_groups=[list(range(NCORES))],
                        ins=[ego_d[:]], outs=[xg[layer][:]])

            nc.vector.tensor_scalar_mul(acc[:], acc[:], 1.0 / NL)
            nc.sync.dma_start(
                out_d[:FULL_RB * 128, :].rearrange("(b p) d -> p b d", p=128),
                acc[:, :FULL_RB, :])
            nc.sync.dma_start(out_d[FULL_RB * 128:, :],
                              acc[0:TAIL_R, FULL_RB, :])
    nc.compile()
    return nc


def _run(inputs, trace=False):
    from concourse.bass_utils import run_bass_kernel_spmd

    user_emb = np.asarray(inputs["user_emb"], dtype=np.float32)
    item_emb = np.asarray(inputs["item_emb"], dtype=np.float32)
    noise = np.asarray(inputs["noise"], dtype=np.float32)
    x0 = np.concatenate([user_emb, item_emb], axis=0)

    seg_tiles, seg_offsets, chunks, T_tot, streams = _preprocess(
        inputs["adj_rows"], inputs["adj_cols"], inputs["adj_vals"])

    nc = _build_program(seg_tiles, seg_offsets, chunks, T_tot)

    in_maps = []
    for k in range(NCORES):
        in_maps.append({
            "x0": x0,
            "idx": streams[k]["idx"],
            "lrw": streams[k]["lrow"],
            "vls": streams[k]["vals"],
            "noise": np.ascontiguousarray(noise[:, k * RPC:(k + 1) * RPC, :]),
        })
    res = run_bass_kernel_spmd(nc, in_maps, core_ids=list(range(NCORES)),
                               trace=trace)
    res._timing_ctx = (nc, in_maps)
    acc = np.concatenate([res.results[k]["out"] for k in range(NCORES)],
                         axis=0)
    user_all = acc[:USER_NUM]
    item_all = acc[USER_NUM:]
    outs = (user_all, item_all,
            np.asarray(inputs["user_prototypes"], dtype=np.float32),
            np.asarray(inputs["item_prototypes"], dtype=np.float32))
    return outs, res


def _time_neff(nc, in_maps, reps=5):
    """Wall-clock the NEFF execution with device-resident inputs.

    Mirrors bass2jax.run_bass_via_pjrt's multi-core path but without
    donation so the same device buffers can be re-executed."""
    import time

    import jax
    import numpy as np_
    from jax.sharding import Mesh, NamedSharding, PartitionSpec
    from jax.experimental.shard_map import shard_map

    import concourse.mybir as mybir
    from concourse import bass2jax

    bass2jax.install_neuronx_cc_hook()

    partition_name = (nc.partition_id_tensor.name
                      if nc.partition_id_tensor else None)
    in_names, out_names, out_avals, zero_outs = [], [], [], []
    for alloc in nc.m.functions[0].allocations:
        if not isinstance(alloc, mybir.MemoryLocationSet):
            continue
        name = alloc.memorylocations[0].name
        if alloc.kind == "ExternalInput":
            if name != partition_name:
                in_names.append(name)
        elif alloc.kind == "ExternalOutput":
            shape = tuple(alloc.tensor_shape)
            dtype = mybir.dt.np(alloc.dtype)
            out_names.append(name)
            out_avals.append(jax.core.ShapedArray(shape, dtype))
            zero_outs.append(np_.zeros(shape, dtype))
    n_params = len(in_names)
    all_names = in_names + out_names

    bind_names = list(all_names)
    if partition_name is not None:
        bind_names.append(partition_name)

    def _body(*args):
        operands = list(args)
        if partition_name is not None:
            operands.append(bass2jax.partition_id_tensor())
        outs = bass2jax._bass_exec_p.bind(
            *operands,
            out_avals=tuple(out_avals),
            in_names=tuple(bind_names),
            out_names=tuple(out_names),
            lowering_input_output_aliases=(),
            sim_require_finite=True,
            sim_require_nnan=True,
            nc=nc,
        )
        return tuple(outs)

    devices = jax.devices()[:NCORES]
    mesh = Mesh(np_.asarray(devices), ("core",))
    nspec = len(all_names)
    sharded = jax.jit(shard_map(
        _body, mesh=mesh, in_specs=(PartitionSpec("core"),) * nspec,
        out_specs=(PartitionSpec("core"),) * len(out_names), check_rep=False))

    sh = NamedSharding(mesh, PartitionSpec("core"))
    dev_args = []
    for i, name in enumerate(all_names):
        if i < n_params:
            arr = np_.concatenate(
                [np_.asarray(m[name]) for m in in_maps], axis=0)
        else:
            z = zero_outs[i - n_params]
            arr = np_.zeros((NCORES * z.shape[0], *z.shape[1:]), z.dtype)
        dev_args.append(jax.device_put(arr, sh))

    times = []
    for _ in range(reps):
        t0 = time.perf_counter()
        out = sharded(*dev_args)
        jax.block_until_ready(out)
        times.append(time.perf_counter() - t0)
    return times


def kernel(**inputs):
    outs, _ = _run(inputs, trace=False)
    return outs
